# revision 7
# baseline (speedup 1.0000x reference)
"""DifferentiableKMeans forward on 8 Trainium2 NeuronCores (Bass/Tile).

Contract: kernel(input_embeddings=[32768,768] f32, centroids=[1024,768] f32)
       -> (clustering_loss: f32 scalar, nearest_centroids: int32 [32768])

Sharding: data-parallel over points (N) across 8 cores; centroids replicated.
Host does layout only (shard + transpose + per-point norms) plus the tiny
cross-core combine: loss = mean_k(S2_k/S1_k) from per-core partial sums
(stable softmin cancels analytically), and an fp64 re-rank of the device's
top-8 argmin candidates for near-tie points (f32r matmul noise).

Device math per core (orientation: points on partitions, centroids on free):
  PSUM P[n,k] = 2048*x.c - 1024*x2[n] - 1024*c2[k] = -1024*d2[n,k]
    - x.c via f32r (rounded-fp32) matmuls, full PE rate
    - x2/c2 folded as 3 extra contraction rows (c2 split hi+lo for accuracy)
  d = sqrt(-P/1024)        ScalarE
  E = exp(-d)   -> bf16    ScalarE
  F = d*E       -> bf16    DVE
  top-8 min d2 + indices:  DVE max/max_index on P (max of -d2)
  S1[k] = sum_n E, S2[k] = sum_n F: ones-vector matmuls accumulated in PSUM
"""
import numpy as np

N_CORES = 8
N, D, K = 32768, 768, 1024
NPTS = N // N_CORES
P = 128
NCHUNK = D // P
GRP = 4
NTILES = NPTS // P
NGROUPS = NTILES // GRP

_COMPILED = {}


def _round_f32r(a):
    """Round fp32 to the PE's fp32r grid: RNE dropping the low 12 mantissa
    bits (verified bit-exact against the on-device cast)."""
    bits = np.ascontiguousarray(a, np.float32).view(np.uint32)
    r = bits + 0x7FF + ((bits >> 12) & 1)
    return (r & np.uint32(0xFFFFF000)).view(np.float32)


def _build():
    import concourse.bacc as bacc
    import concourse.mybir as mybir
    from concourse.alu_op_type import AluOpType
    from concourse.tile import TileContext

    F32 = mybir.dt.float32
    F32R = mybir.dt.float32r
    BF16 = mybir.dt.bfloat16
    U32 = mybir.dt.uint32
    AF = mybir.ActivationFunctionType

    nc = bacc.Bacc("TRN2", target_bir_lowering=False, debug=False,
                   num_devices=N_CORES)

    xt_d = nc.declare_dram_parameter("xt", [D, NPTS], F32R, isOutput=False)
    x2_d = nc.declare_dram_parameter("x2", [1, NPTS], F32, isOutput=False)
    ct_d = nc.declare_dram_parameter("ct", [D, K], F32, isOutput=False)
    idx_d = nc.declare_dram_parameter("idx", [NPTS, 8], U32, isOutput=True)
    val_d = nc.declare_dram_parameter("val", [NPTS, 8], F32, isOutput=True)
    s1_d = nc.declare_dram_parameter("s1", [1, K], F32, isOutput=True)
    s2_d = nc.declare_dram_parameter("s2", [1, K], F32, isOutput=True)

    with TileContext(nc) as tc:
        with tc.tile_pool(name="const", bufs=1) as const, \
             tc.tile_pool(name="xtp", bufs=2) as xtp, \
             tc.tile_pool(name="work", bufs=3) as work, \
             tc.tile_pool(name="stage", bufs=1) as stage, \
             tc.tile_pool(name="ppool", bufs=2, space="PSUM") as ppool, \
             tc.tile_pool(name="accp", bufs=1, space="PSUM") as accp:

            # ---------- setup ----------
            ones_f = const.tile([P, 1], F32, name="ones_f")
            nc.gpsimd.memset(ones_f[:], 1.0)
            ones_bf = const.tile([P, 1], BF16, name="ones_bf")
            nc.gpsimd.memset(ones_bf[:], 1.0)

            # per-chunk: load CT chunk, make scaled f32r copy, square for c2
            ct_view = ct_d[:].rearrange("(c p) k -> p c k", p=P)
            ct_r = const.tile([P, NCHUNK, K], F32R, name="ct_r")
            c2ps = accp.tile([1, K], F32, name="c2ps", tag="accB")
            for c in range(NCHUNK):
                cld = work.tile([P, K], F32, name="cld", tag="cload", bufs=2)
                nc.sync.dma_start(out=cld[:], in_=ct_view[:, c, :])
                ct_s = work.tile([P, K], F32, name="ct_s", tag="ct_s", bufs=2)
                nc.vector.tensor_scalar(ct_s[:], cld[:], 2048.0, None,
                                        AluOpType.mult)
                nc.vector.tensor_copy(ct_r[:, c, :], ct_s[:])
                sq = work.tile([P, K], F32, name="sq", tag="sq", bufs=2)
                nc.scalar.activation(sq[:], cld[:], AF.Square)
                for h in range(2):
                    nc.tensor.matmul(c2ps[0:1, h * 512:(h + 1) * 512], ones_f[:],
                                     sq[:, h * 512:(h + 1) * 512],
                                     start=(c == 0), stop=(c == NCHUNK - 1),
                                     skip_group_check=True)

            # extras rhs [3, K]: row0 = 1, row1 = -1024*c2_hi, row2 = -1024*c2_lo
            t1 = stage.tile([1, K], F32, name="t1", tag="rowK", bufs=4)
            nc.vector.tensor_scalar(t1[:], c2ps[0:1, :], -1024.0, None, AluOpType.mult)
            c2hi = stage.tile([1, K], F32R, name="c2hi", tag="rowK", bufs=4)
            nc.vector.tensor_copy(c2hi[:], t1[:])
            t2 = stage.tile([1, K], F32, name="t2", tag="rowK", bufs=4)
            nc.vector.tensor_sub(t2[:], t1[:], c2hi[:].bitcast(F32))
            c2lo = stage.tile([1, K], F32R, name="c2lo", tag="rowK", bufs=4)
            nc.vector.tensor_copy(c2lo[:], t2[:])
            ex_r = const.tile([3, K], F32R, name="ex_r")
            nc.gpsimd.memset(ex_r[:].bitcast(F32), 1.0)
            nc.sync.dma_start(out=ex_r[1:2, :], in_=c2hi[:])
            nc.sync.dma_start(out=ex_r[2:3, :], in_=c2lo[:])

            # extras lhsT [3, NPTS]: row0 = -1024*x2, rows 1,2 = 1
            x2row = stage.tile([1, NPTS], F32, name="x2row", tag="row16", bufs=2)
            nc.sync.dma_start(out=x2row[:], in_=x2_d[:])
            x2n_f = stage.tile([1, NPTS], F32, name="x2n_f", tag="row16", bufs=2)
            nc.vector.tensor_scalar(x2n_f[:], x2row[0:1, :], -1024.0, None,
                                    AluOpType.mult)
            x2n_r = stage.tile([1, NPTS], F32R, name="x2n_r", tag="row16", bufs=2)
            nc.vector.tensor_copy(x2n_r[:], x2n_f[:])
            ex_l = const.tile([3, NPTS], F32R, name="ex_l")
            nc.gpsimd.memset(ex_l[:].bitcast(F32), 1.0)
            nc.sync.dma_start(out=ex_l[0:1, :], in_=x2n_r[:])

            idx_st = stage.tile([P, NTILES, 8], U32, name="idx_st")
            val_st = stage.tile([P, NTILES, 8], F32, name="val_st")
            s1ps = accp.tile([1, K], F32, name="s1ps", tag="accA")
            s2ps = accp.tile([1, K], F32, name="s2ps", tag="accB")

            xt_view = xt_d[:].rearrange("(c p) n -> p c n", p=P)

            for g in range(NGROUPS):
                xt_g = xtp.tile([P, NCHUNK, GRP * P], F32R, name="xt_g")
                nc.sync.dma_start(out=xt_g[:],
                                  in_=xt_view[:, :, g * GRP * P:(g + 1) * GRP * P])

                for j in range(GRP):
                    t = g * GRP + j
                    pt = ppool.tile([P, K], F32, name="pt")
                    for c in range(NCHUNK):
                        for h in range(2):
                            nc.tensor.matmul(pt[:, h * 512:(h + 1) * 512],
                                             xt_g[:, c, j * P:(j + 1) * P],
                                             ct_r[:, c, h * 512:(h + 1) * 512],
                                             start=(c == 0), stop=False)
                    for h in range(2):
                        nc.tensor.matmul(pt[:, h * 512:(h + 1) * 512],
                                         ex_l[:, t * P:(t + 1) * P],
                                         ex_r[:, h * 512:(h + 1) * 512],
                                         start=False, stop=True)

                    # d = sqrt(-P/1024) computed as exp(0.5*ln(-P/1024)) so the
                    # whole kernel stays inside one ACT table set (ln+exp+square)
                    u_t = work.tile([P, K], F32, name="u_t")
                    nc.scalar.activation(u_t[:], pt[:], AF.Ln, scale=-1.0 / 1024.0)
                    d_t = work.tile([P, K], F32, name="d_t")
                    nc.scalar.activation(d_t[:], u_t[:], AF.Exp, scale=0.5)
                    e_bf = work.tile([P, K], BF16, name="e_bf")
                    nc.scalar.activation(e_bf[:], d_t[:], AF.Exp, scale=-1.0)
                    f_bf = work.tile([P, K], BF16, name="f_bf")
                    nc.gpsimd.tensor_tensor(f_bf[:], d_t[:], e_bf[:], AluOpType.mult)

                    nc.vector.max(val_st[:, t, :], pt[:])
                    nc.vector.max_index(idx_st[:, t, :], val_st[:, t, :], pt[:])

                    for h in range(2):
                        nc.tensor.matmul(s1ps[0:1, h * 512:(h + 1) * 512], ones_bf[:],
                                         e_bf[:, h * 512:(h + 1) * 512],
                                         start=(t == 0), stop=(t == NTILES - 1),
                                         skip_group_check=True)
                        nc.tensor.matmul(s2ps[0:1, h * 512:(h + 1) * 512], ones_bf[:],
                                         f_bf[:, h * 512:(h + 1) * 512],
                                         start=(t == 0), stop=(t == NTILES - 1),
                                         skip_group_check=True)

            # ---------- drain ----------
            s1sb = stage.tile([1, K], F32, name="s1sb")
            s2sb = stage.tile([1, K], F32, name="s2sb")
            nc.scalar.copy(out=s1sb[:], in_=s1ps[:])
            nc.scalar.copy(out=s2sb[:], in_=s2ps[:])
            nc.sync.dma_start(out=s1_d[:], in_=s1sb[:])
            nc.sync.dma_start(out=s2_d[:], in_=s2sb[:])
            nc.sync.dma_start(out=idx_d[:].rearrange("(t p) j -> p t j", p=P),
                              in_=idx_st[:])
            nc.sync.dma_start(out=val_d[:].rearrange("(t p) j -> p t j", p=P),
                              in_=val_st[:])

    nc.compile()
    return nc


def _get_compiled():
    if "nc" not in _COMPILED:
        _COMPILED["nc"] = _build()
    return _COMPILED["nc"]


def kernel(input_embeddings, centroids):
    from concourse.bass_utils import run_bass_kernel_spmd

    X = np.ascontiguousarray(np.asarray(input_embeddings, dtype=np.float32))
    C = np.ascontiguousarray(np.asarray(centroids, dtype=np.float32))
    assert X.shape == (N, D) and C.shape == (K, D)

    nc = _get_compiled()

    CT = np.ascontiguousarray(C.T)
    XT_r = _round_f32r(X.T)  # pre-round to the PE's fp32r grid (layout prep)
    in_maps = []
    for i in range(N_CORES):
        Xs = X[i * NPTS:(i + 1) * NPTS]
        in_maps.append({
            "xt": np.ascontiguousarray(XT_r[:, i * NPTS:(i + 1) * NPTS]),
            "x2": np.einsum('nd,nd->n', Xs, Xs, dtype=np.float64)
                    .astype(np.float32)[None, :],
            "ct": CT,
        })

    res = run_bass_kernel_spmd(nc, in_maps, list(range(N_CORES)))

    # ---- host combine (fp64) ----
    S1 = np.zeros(K, np.float64)
    S2 = np.zeros(K, np.float64)
    nearest = np.empty(N, np.int64)
    idx_all = np.empty((N, 8), np.int64)
    val_all = np.empty((N, 8), np.float64)
    for i in range(N_CORES):
        r = res.results[i]
        S1 += r["s1"][0].astype(np.float64)
        S2 += r["s2"][0].astype(np.float64)
        idx_all[i * NPTS:(i + 1) * NPTS] = r["idx"].astype(np.int64)
        val_all[i * NPTS:(i + 1) * NPTS] = r["val"].astype(np.float64)

    loss = np.float32((S2 / S1).mean())

    nearest = idx_all[:, 0].copy()
    # near-tie fixup: device distances carry ~0.01 d2 noise (f32r matmuls);
    # re-rank the top-8 candidates in fp64 where the top-2 gap is small.
    gap = (val_all[:, 0] - val_all[:, 1]) / 1024.0   # d2 gap, top1 vs top2
    flagged = np.nonzero(gap < 0.25)[0]
    if flagged.size:
        X64 = X.astype(np.float64)
        C64 = C.astype(np.float64)
        c2_64 = np.einsum('kd,kd->k', C64, C64)
        for n in flagged:
            cand = idx_all[n]
            d2c = (X64[n] @ X64[n]) + c2_64[cand] - 2.0 * (C64[cand] @ X64[n])
            nearest[n] = cand[np.argmin(d2c)]

    return loss, nearest.astype(np.int32)


# revision 12
# speedup vs baseline: 1.2217x; 1.2217x over previous
"""DifferentiableKMeans forward on 8 Trainium2 NeuronCores (Bass/Tile).

Contract: kernel(input_embeddings=[32768,768] f32, centroids=[1024,768] f32)
       -> (clustering_loss: f32 scalar, nearest_centroids: int32 [32768])

Sharding: data-parallel over points (N) across 8 cores; centroids replicated.
Host does layout only (shard + transpose + per-point norms) plus the tiny
cross-core combine: loss = mean_k(S2_k/S1_k) from per-core partial sums
(stable softmin cancels analytically), and an fp64 re-rank of the device's
top-8 argmin candidates for near-tie points (f32r matmul noise).

Device math per core (orientation: points on partitions, centroids on free):
  PSUM P[n,k] = 2048*x.c - 1024*x2[n] - 1024*c2[k] = -1024*d2[n,k]
    - x.c via f32r (rounded-fp32) matmuls, full PE rate
    - x2/c2 folded as 3 extra contraction rows (c2 split hi+lo for accuracy)
  d = sqrt(-P/1024)        ScalarE
  E = exp(-d)   -> bf16    ScalarE
  F = d*E       -> bf16    DVE
  top-8 min d2 + indices:  DVE max/max_index on P (max of -d2)
  S1[k] = sum_n E, S2[k] = sum_n F: ones-vector matmuls accumulated in PSUM
"""
import os
import numpy as np

N_CORES = 8
N, D, K = 32768, 768, 1024
NPTS = N // N_CORES
P = 128
NCHUNK = D // P
GRP = 4
NTILES = NPTS // P
NGROUPS = NTILES // GRP

_COMPILED = {}


def _round_f32r(a):
    """Round fp32 to the PE's fp32r grid: RNE dropping the low 12 mantissa
    bits (verified bit-exact against the on-device cast)."""
    bits = np.ascontiguousarray(a, np.float32).view(np.uint32)
    r = bits + 0x7FF + ((bits >> 12) & 1)
    return (r & np.uint32(0xFFFFF000)).view(np.float32)


def _setup_act_root():
    """Point walrus at an act-table root whose only set is
    natural_log_exp_and_others (exp+ln+square+copy), so every ScalarE
    function resolves to one table set and no per-tile reloads happen."""
    import json, shutil, neuronxcc
    dst = "/tmp/dkm_act_root"
    marker = os.path.join(dst, "act_info.json")
    if not os.path.exists(marker):
        src = os.path.join(os.path.dirname(neuronxcc.__file__),
                           "pwp", "pwp_bin_trainium")
        shutil.copytree(src, dst, dirs_exist_ok=True)
        with open(os.path.join(src, "act_info.json")) as f:
            info = json.load(f)
        keep = [s for s in info["act_func_sets"]
                if s["name"] == "natural_log_exp_and_others"]
        assert keep, "natural_log_exp_and_others set missing from act_info"
        info["act_func_sets"] = keep
        with open(marker, "w") as f:
            json.dump(info, f)
    os.environ["BASS_ACT_ROOT_JSON_PATH"] = marker

    # keep the bass-side ATL placement consistent with the 1-set root
    import concourse.hw_specs as hw_specs
    import concourse.bacc as bacc_mod
    orig = hw_specs.get_activation_tables
    if not getattr(hw_specs, "_dkm_patched", False):
        def only_nle(arch, _orig=orig):
            t = _orig(arch)
            return {"natural_log_exp_and_others": t["natural_log_exp_and_others"]}
        hw_specs.get_activation_tables = only_nle
        hw_specs._dkm_patched = True
        if getattr(bacc_mod, "get_activation_tables", None) is not None:
            bacc_mod.get_activation_tables = only_nle


def _build():
    import concourse.bacc as bacc
    import concourse.mybir as mybir
    from concourse.alu_op_type import AluOpType
    from concourse.tile import TileContext

    F32 = mybir.dt.float32
    F32R = mybir.dt.float32r
    BF16 = mybir.dt.bfloat16
    U32 = mybir.dt.uint32
    AF = mybir.ActivationFunctionType

    nc = bacc.Bacc("TRN2", target_bir_lowering=False, debug=False,
                   num_devices=N_CORES)

    xt_d = nc.declare_dram_parameter("xt", [D, NPTS], F32R, isOutput=False)
    x2_d = nc.declare_dram_parameter("x2", [1, NPTS], F32, isOutput=False)
    ct_d = nc.declare_dram_parameter("ct", [D, K], F32, isOutput=False)
    idx_d = nc.declare_dram_parameter("idx", [NPTS, 8], U32, isOutput=True)
    val_d = nc.declare_dram_parameter("val", [NPTS, 8], F32, isOutput=True)
    s1_d = nc.declare_dram_parameter("s1", [1, K], F32, isOutput=True)
    s2_d = nc.declare_dram_parameter("s2", [1, K], F32, isOutput=True)

    with TileContext(nc) as tc:
        with tc.tile_pool(name="const", bufs=1) as const, \
             tc.tile_pool(name="xtp", bufs=2) as xtp, \
             tc.tile_pool(name="work", bufs=3) as work, \
             tc.tile_pool(name="stage", bufs=1) as stage, \
             tc.tile_pool(name="ppool", bufs=2, space="PSUM") as ppool, \
             tc.tile_pool(name="accp", bufs=1, space="PSUM") as accp:

            # ---------- setup ----------
            ones_f = const.tile([P, 1], F32, name="ones_f")
            nc.gpsimd.memset(ones_f[:], 1.0)
            ones_bf = const.tile([P, 1], BF16, name="ones_bf")
            nc.gpsimd.memset(ones_bf[:], 1.0)

            # per-chunk: load CT chunk, make scaled f32r copy, square for c2
            ct_view = ct_d[:].rearrange("(c p) k -> p c k", p=P)
            ct_r = const.tile([P, NCHUNK, K], F32R, name="ct_r")
            c2ps = accp.tile([1, K], F32, name="c2ps", tag="accB")
            for c in range(NCHUNK):
                cld = work.tile([P, K], F32, name="cld", tag="cload", bufs=2)
                nc.sync.dma_start(out=cld[:], in_=ct_view[:, c, :])
                ct_s = work.tile([P, K], F32, name="ct_s", tag="ct_s", bufs=2)
                nc.vector.tensor_scalar(ct_s[:], cld[:], 2048.0, None,
                                        AluOpType.mult)
                nc.vector.tensor_copy(ct_r[:, c, :], ct_s[:])
                sq = work.tile([P, K], F32, name="sq", tag="sq", bufs=2)
                nc.scalar.activation(sq[:], cld[:], AF.Square)
                for h in range(2):
                    nc.tensor.matmul(c2ps[0:1, h * 512:(h + 1) * 512], ones_f[:],
                                     sq[:, h * 512:(h + 1) * 512],
                                     start=(c == 0), stop=(c == NCHUNK - 1),
                                     skip_group_check=True)

            # extras rhs [3, K]: row0 = 1, row1 = -1024*c2_hi, row2 = -1024*c2_lo
            t1 = stage.tile([1, K], F32, name="t1", tag="rowK", bufs=4)
            nc.vector.tensor_scalar(t1[:], c2ps[0:1, :], -1024.0, None, AluOpType.mult)
            c2hi = stage.tile([1, K], F32R, name="c2hi", tag="rowK", bufs=4)
            nc.vector.tensor_copy(c2hi[:], t1[:])
            t2 = stage.tile([1, K], F32, name="t2", tag="rowK", bufs=4)
            nc.vector.tensor_sub(t2[:], t1[:], c2hi[:].bitcast(F32))
            c2lo = stage.tile([1, K], F32R, name="c2lo", tag="rowK", bufs=4)
            nc.vector.tensor_copy(c2lo[:], t2[:])
            ex_r = const.tile([3, K], F32R, name="ex_r")
            nc.gpsimd.memset(ex_r[:].bitcast(F32), 1.0)
            nc.sync.dma_start(out=ex_r[1:2, :], in_=c2hi[:])
            nc.sync.dma_start(out=ex_r[2:3, :], in_=c2lo[:])

            # extras lhsT [3, NPTS]: row0 = -1024*x2, rows 1,2 = 1
            x2row = stage.tile([1, NPTS], F32, name="x2row", tag="row16", bufs=2)
            nc.sync.dma_start(out=x2row[:], in_=x2_d[:])
            x2n_f = stage.tile([1, NPTS], F32, name="x2n_f", tag="row16", bufs=2)
            nc.vector.tensor_scalar(x2n_f[:], x2row[0:1, :], -1024.0, None,
                                    AluOpType.mult)
            x2n_r = stage.tile([1, NPTS], F32R, name="x2n_r", tag="row16", bufs=2)
            nc.vector.tensor_copy(x2n_r[:], x2n_f[:])
            ex_l = const.tile([3, NPTS], F32R, name="ex_l")
            nc.gpsimd.memset(ex_l[:].bitcast(F32), 1.0)
            nc.sync.dma_start(out=ex_l[0:1, :], in_=x2n_r[:])

            idx_st = stage.tile([P, NTILES, 8], U32, name="idx_st")
            val_st = stage.tile([P, NTILES, 8], F32, name="val_st")
            s1ps = accp.tile([1, K], F32, name="s1ps", tag="accA")
            s2ps = accp.tile([1, K], F32, name="s2ps", tag="accB")

            xt_view = xt_d[:].rearrange("(c p) n -> p c n", p=P)

            for g in range(NGROUPS):
                xt_g = xtp.tile([P, NCHUNK, GRP * P], F32R, name="xt_g")
                nc.sync.dma_start(out=xt_g[:],
                                  in_=xt_view[:, :, g * GRP * P:(g + 1) * GRP * P])

                for j in range(GRP):
                    t = g * GRP + j
                    pt = ppool.tile([P, K], F32, name="pt")
                    for c in range(NCHUNK):
                        for h in range(2):
                            nc.tensor.matmul(pt[:, h * 512:(h + 1) * 512],
                                             xt_g[:, c, j * P:(j + 1) * P],
                                             ct_r[:, c, h * 512:(h + 1) * 512],
                                             start=(c == 0), stop=False)
                    for h in range(2):
                        nc.tensor.matmul(pt[:, h * 512:(h + 1) * 512],
                                         ex_l[:, t * P:(t + 1) * P],
                                         ex_r[:, h * 512:(h + 1) * 512],
                                         start=False, stop=True)

                    # d = sqrt(-P/1024) computed as exp(0.5*ln(-P/1024)) so the
                    # whole kernel stays inside one ACT table set (ln+exp+square)
                    u_t = work.tile([P, K], F32, name="u_t")
                    nc.scalar.activation(u_t[:], pt[:], AF.Ln, scale=-1.0 / 1024.0)
                    d_t = work.tile([P, K], F32, name="d_t")
                    nc.scalar.activation(d_t[:], u_t[:], AF.Exp, scale=0.5)
                    e_bf = work.tile([P, K], BF16, name="e_bf")
                    nc.scalar.activation(e_bf[:], d_t[:], AF.Exp, scale=-1.0)
                    f_bf = work.tile([P, K], BF16, name="f_bf")
                    nc.vector.tensor_tensor(f_bf[:], d_t[:], e_bf[:], AluOpType.mult)

                    nc.vector.max(val_st[:, t, :], pt[:])
                    nc.vector.max_index(idx_st[:, t, :], val_st[:, t, :], pt[:])

                    for h in range(2):
                        nc.tensor.matmul(s1ps[0:1, h * 512:(h + 1) * 512], ones_bf[:],
                                         e_bf[:, h * 512:(h + 1) * 512],
                                         start=(t == 0), stop=(t == NTILES - 1),
                                         skip_group_check=True)
                        nc.tensor.matmul(s2ps[0:1, h * 512:(h + 1) * 512], ones_bf[:],
                                         f_bf[:, h * 512:(h + 1) * 512],
                                         start=(t == 0), stop=(t == NTILES - 1),
                                         skip_group_check=True)

            # ---------- drain ----------
            s1sb = stage.tile([1, K], F32, name="s1sb")
            s2sb = stage.tile([1, K], F32, name="s2sb")
            nc.scalar.copy(out=s1sb[:], in_=s1ps[:])
            nc.scalar.copy(out=s2sb[:], in_=s2ps[:])
            nc.sync.dma_start(out=s1_d[:], in_=s1sb[:])
            nc.sync.dma_start(out=s2_d[:], in_=s2sb[:])
            nc.sync.dma_start(out=idx_d[:].rearrange("(t p) j -> p t j", p=P),
                              in_=idx_st[:])
            nc.sync.dma_start(out=val_d[:].rearrange("(t p) j -> p t j", p=P),
                              in_=val_st[:])

    nc.compile()
    return nc


def _get_compiled():
    if "nc" not in _COMPILED:
        _setup_act_root()
        _COMPILED["nc"] = _build()
    return _COMPILED["nc"]


def kernel(input_embeddings, centroids):
    from concourse.bass_utils import run_bass_kernel_spmd

    X = np.ascontiguousarray(np.asarray(input_embeddings, dtype=np.float32))
    C = np.ascontiguousarray(np.asarray(centroids, dtype=np.float32))
    assert X.shape == (N, D) and C.shape == (K, D)

    nc = _get_compiled()

    CT = np.ascontiguousarray(C.T)
    XT_r = _round_f32r(X.T)  # pre-round to the PE's fp32r grid (layout prep)
    in_maps = []
    for i in range(N_CORES):
        Xs = X[i * NPTS:(i + 1) * NPTS]
        in_maps.append({
            "xt": np.ascontiguousarray(XT_r[:, i * NPTS:(i + 1) * NPTS]),
            "x2": np.einsum('nd,nd->n', Xs, Xs, dtype=np.float64)
                    .astype(np.float32)[None, :],
            "ct": CT,
        })

    res = run_bass_kernel_spmd(nc, in_maps, list(range(N_CORES)))

    # ---- host combine (fp64) ----
    S1 = np.zeros(K, np.float64)
    S2 = np.zeros(K, np.float64)
    nearest = np.empty(N, np.int64)
    idx_all = np.empty((N, 8), np.int64)
    val_all = np.empty((N, 8), np.float64)
    for i in range(N_CORES):
        r = res.results[i]
        S1 += r["s1"][0].astype(np.float64)
        S2 += r["s2"][0].astype(np.float64)
        idx_all[i * NPTS:(i + 1) * NPTS] = r["idx"].astype(np.int64)
        val_all[i * NPTS:(i + 1) * NPTS] = r["val"].astype(np.float64)

    loss = np.float32((S2 / S1).mean())

    nearest = idx_all[:, 0].copy()
    # near-tie fixup: device distances carry ~0.01 d2 noise (f32r matmuls);
    # re-rank the top-8 candidates in fp64 where the top-2 gap is small.
    gap = (val_all[:, 0] - val_all[:, 1]) / 1024.0   # d2 gap, top1 vs top2
    flagged = np.nonzero(gap < 0.25)[0]
    if flagged.size:
        X64 = X.astype(np.float64)
        C64 = C.astype(np.float64)
        c2_64 = np.einsum('kd,kd->k', C64, C64)
        for n in flagged:
            cand = idx_all[n]
            d2c = (X64[n] @ X64[n]) + c2_64[cand] - 2.0 * (C64[cand] @ X64[n])
            nearest[n] = cand[np.argmin(d2c)]

    return loss, nearest.astype(np.int32)
